# revision 17
# baseline (speedup 1.0000x reference)
"""MultiHeadLinearAttention (Linformer-style) on 8 trn2 NeuronCores.

Strategy (head-parallel attention + per-batch AllToAll + token-parallel
output projection):
  - 16 heads -> 8 cores: 2 heads (one d_model slice of 128) per core.
  - Per core, per batch b:
      Kp  [128(d2), 256k]  = K_slice^T @ We             (+be)
      Vp  [256(k), 128(d2)] = Wf^T @ V_slice            (+bf)
      per head h, 512-token block nh:
        s^T [256k, 512n] = Kp_h^T @ Q_h^T   (64-row PE tile, no padding)
        E^T = exp(s^T / 8)                  (one 1024-wide Act instr)
        at  [128, 512] = [Vp_h | ONES]^T @ E^T
            rows 0:64  = unnormalized attention output^T
            rows 64:128 = softmax denominator REPLICATED 64x (free PE
            broadcast -> plain elementwise normalize, no partition
            broadcast machinery)
        attn^T = at[0:64] * reciprocal(at[64:128])  (DVE recip + Pool mul)
  - Per-batch AllToAll (4 small collectives, pipelined with compute)
    exchanges attn^T n-chunks so core i ends with ALL d_model rows for
    tokens [512*i, 512*(i+1)) of that batch.
  - Per core, per b: out[n_shard] = attn_full^T.T @ Wo (+bo).

All matmuls in bf16 (host-cast) with fp32 PSUM accumulation.  Host
pre-arranges all DRAM tensors so every DMA is >=1KB-contiguous per
partition line, and DMAs are spread across the per-engine hardware DGE
queues (sync: K/V + a2a_out, vector: We/Wf + a2a_in, scalar: Q^T + Wo +
output stores) to avoid head-of-line blocking.
"""

import numpy as np
import ml_dtypes

import concourse.bass as bass
import concourse.mybir as mybir
from concourse.tile import TileContext
from concourse.bass_utils import run_bass_kernel_spmd

B, N, D, H, LK = 4, 4096, 1024, 16, 256
DK = D // H          # 64
NC = 8               # cores
NSH = N // NC        # 512 tokens per (core, nh-block)
P = 128
NCH = 32             # 128-row chunks of N
NPH = 8              # 512-col chunks of N

F32 = mybir.dt.float32
BF16 = mybir.dt.bfloat16
NP_BF16 = ml_dtypes.bfloat16

_BUILD_CACHE = {}

_ws_ctr = [0]


def _split_multi_waits(nc, lim=1):
    """Walrus codegen on this stack rejects instructions whose on_wait list
    exceeds the per-format wait-slot count ("Too many sync wait commands").
    Engines execute in order, so excess waits move onto preceding NOPs on
    the same engine with identical semantics."""
    for f in nc.m.functions:
        for blk in f.blocks:
            insts = blk.instructions
            if not any(
                ins.sync_info is not None and len(ins.sync_info.on_wait or []) > lim
                for ins in insts
            ):
                continue
            out = []
            for ins in insts:
                si = ins.sync_info
                waits = list(si.on_wait) if si is not None and si.on_wait else []
                if len(waits) > lim and ins.engine is not None:
                    keep = waits[-lim:]
                    rest = waits[:-lim]
                    while rest:
                        chunk, rest = rest[:lim], rest[lim:]
                        _ws_ctr[0] += 1
                        nop = mybir.InstNoOp(
                            name=f"I-waitsplit-{_ws_ctr[0]}", ins=[], outs=[]
                        )
                        nop.engine = ins.engine
                        nop.sync_info = mybir.SyncInfo(on_wait=chunk, on_update=[])
                        out.append(nop)
                    ins.sync_info = mybir.SyncInfo(
                        on_wait=keep, on_update=list(si.on_update or [])
                    )
                out.append(ins)
            blk.instructions = out
    return nc


def _build(use_be, use_bf, use_bo):
    nc = bass.Bass(num_devices=NC)

    Ks_p = nc.declare_dram_parameter("Ks", [N, B, P], BF16, isOutput=False)
    Vs_p = nc.declare_dram_parameter("Vs", [N, B, P], BF16, isOutput=False)
    QT_p = nc.declare_dram_parameter("QTs", [B, P, N], BF16, isOutput=False)
    We_p = nc.declare_dram_parameter("WeS", [P, NCH, LK], BF16, isOutput=False)
    Wf_p = nc.declare_dram_parameter("WfS", [P, NCH, LK], BF16, isOutput=False)
    Wo_p = nc.declare_dram_parameter("WoS", [P, D // P, D], BF16, isOutput=False)
    if use_be:
        be_p = nc.declare_dram_parameter("beB", [P, LK], F32, isOutput=False)
    if use_bf:
        bf_p = nc.declare_dram_parameter("bfB", [P, 2], F32, isOutput=False)
    if use_bo:
        bo_p = nc.declare_dram_parameter("boB", [P, D], F32, isOutput=False)
    out_p = nc.declare_dram_parameter("out", [B, NSH, D], F32, isOutput=True)

    rg = [list(range(NC))]

    with TileContext(nc) as tc:
        with (
            tc.tile_pool(name="wpool", bufs=1) as wpool,
            tc.tile_pool(name="state", bufs=1) as state,
            tc.tile_pool(name="dram", bufs=1, space="DRAM") as dram,
        ):
            # ---- persistent tiles
            We_sb = wpool.tile([P, NCH, LK], BF16)
            Wf_sb = wpool.tile([P, NCH, LK], BF16)
            Wo_sb = wpool.tile([P, D // P, D], BF16)
            # kp_sb[d2(=2 heads x 64), b, kc, k128]
            kp_sb = state.tile([P, B, 2, P], BF16)
            # vp_aug[k128, kc, b, h, (d64 | ones64)]
            vp_aug = state.tile([P, 2, B, 2, P], BF16)
            nc.gpsimd.memset(vp_aug[:, :, :, :, DK:], 1.0)

            # We/Wf chunks interleave with K/V loads on the sync queue so
            # the first phase-1 matmul can start after ~3 transfers.
            if use_be:
                be_sb = wpool.tile([P, LK], F32)
                nc.scalar.dma_start(be_sb[:], be_p[:])
            if use_bf:
                bf_sb = wpool.tile([P, 2], F32)
                nc.scalar.dma_start(bf_sb[:], bf_p[:])

            # ---- per-batch A2A buffers
            a2a_in = [
                dram.tile([NC, P, NSH], BF16, name=f"a2a_in{b}") for b in range(B)
            ]
            a2a_out = [
                dram.tile([NC, P, NSH], BF16, name=f"a2a_out{b}") for b in range(B)
            ]

            # ================= phase 1: Kp / Vp =================
            with (
                tc.tile_pool(name="p1", bufs=1) as p1,
                tc.tile_pool(name="p1ps", bufs=1, space="PSUM") as p1ps,
            ):
                kp_ps = [
                    p1ps.tile([P, LK], F32, name=f"kp{b}", tag=f"kp{b}")
                    for b in range(B)
                ]
                vp_ps = [
                    p1ps.tile([P, B * P], F32, name=f"vp{kc}", tag=f"vp{kc}")
                    for kc in range(2)
                ]
                cs = slice(0, 4)
                nc.sync.dma_start(We_sb[:, cs, :], We_p[:, cs, :])
                nc.sync.dma_start(Wf_sb[:, cs, :], Wf_p[:, cs, :])
                for ic2 in range(NCH // 2):
                    if ic2 % 2 == 0 and ic2 // 2 + 1 < NCH // 4:
                        # prefetch the next 4-chunk slab of We/Wf one slab
                        # ahead of its first use
                        ch = ic2 // 2 + 1
                        cs = slice(ch * 4, (ch + 1) * 4)
                        nc.sync.dma_start(We_sb[:, cs, :], We_p[:, cs, :])
                        nc.sync.dma_start(Wf_sb[:, cs, :], Wf_p[:, cs, :])
                    K4 = p1.tile([P, 2, B, P], BF16, name="K4", tag="K4", bufs=3)
                    nc.sync.dma_start(
                        K4[:],
                        Ks_p[ic2 * 2 * P : (ic2 + 1) * 2 * P, :, :].rearrange(
                            "(i p) b d -> p i b d", p=P
                        ),
                    )
                    V4 = p1.tile([P, 2, B, P], BF16, name="V4", tag="V4", bufs=3)
                    nc.sync.dma_start(
                        V4[:],
                        Vs_p[ic2 * 2 * P : (ic2 + 1) * 2 * P, :, :].rearrange(
                            "(i p) b d -> p i b d", p=P
                        ),
                    )
                    for i in range(2):
                        ic = ic2 * 2 + i
                        for b in range(B):
                            nc.tensor.matmul(
                                kp_ps[b][:],
                                K4[:, i, b, :],
                                We_sb[:, ic, :],
                                start=(ic == 0),
                                stop=(ic == NCH - 1),
                            )
                        for kc in range(2):
                            nc.tensor.matmul(
                                vp_ps[kc][:],
                                Wf_sb[:, ic, kc * P : (kc + 1) * P],
                                V4[:, i, :, :],
                                start=(ic == 0),
                                stop=(ic == NCH - 1),
                            )

                # epilogue: stage Kp (DVE) and Vp (Pool) into bf16 SBUF
                for b in range(B):
                    for kc in range(2):
                        ks = slice(kc * P, (kc + 1) * P)
                        if use_be:
                            nc.vector.tensor_tensor(
                                kp_sb[:, b, kc, :],
                                kp_ps[b][:, ks],
                                be_sb[:, ks],
                                mybir.AluOpType.add,
                            )
                        else:
                            nc.vector.tensor_copy(kp_sb[:, b, kc, :], kp_ps[b][:, ks])
                    for kc in range(2):
                        for h in range(2):
                            src = vp_ps[kc][:, b * P + h * DK : b * P + (h + 1) * DK]
                            dst = vp_aug[:, kc, b, h, 0:DK]
                            if use_bf:
                                nc.vector.tensor_scalar_add(
                                    dst, src, bf_sb[:, kc : kc + 1]
                                )
                            else:
                                nc.vector.tensor_copy(dst, src)

            # ================= phase 2: scores/softmax/attn + per-b A2A ====
            with (
                tc.tile_pool(name="p2", bufs=1) as p2,
                tc.tile_pool(name="p2ps", bufs=1, space="PSUM") as p2ps,
                tc.tile_pool(name="p3", bufs=1) as p3,
                tc.tile_pool(name="p3ps", bufs=1, space="PSUM") as p3ps,
            ):
                for b in range(B):
                    for nh in range(NPH):
                        if nh % 2 == 0:
                            QT2 = p2.tile(
                                [P, 2, NSH], BF16, name="QT2", tag="QT2", bufs=3
                            )
                            nc.scalar.dma_start(
                                QT2[:], QT_p[b, :, nh * NSH : (nh + 2) * NSH]
                            )
                        attn_sb = p2.tile(
                            [P, NSH], BF16, name="attn", tag="attn", bufs=3
                        )
                        for h in range(2):
                            # head h of this core's 128-wide d_model slice
                            # lives on partitions h*64 .. h*64+64
                            hp = slice(h * DK, (h + 1) * DK)
                            st = p2ps.tile(
                                [P, 2, NSH], F32, name="st", tag="st", bufs=2
                            )
                            for kc in range(2):
                                nc.tensor.matmul(
                                    st[:, kc, :],
                                    kp_sb[hp, b, kc, :],
                                    QT2[hp, nh % 2, :],
                                    start=True,
                                    stop=True,
                                )
                            ET = p2.tile([P, 2, NSH], BF16, name="ET", tag="ET",
                                         bufs=2)
                            nc.scalar.activation(
                                ET[:, :, :],
                                st[:, :, :],
                                mybir.ActivationFunctionType.Exp,
                                scale=0.125,
                            )
                            at2 = p2ps.tile([P, NSH], F32, name="at", tag="at",
                                            bufs=2)
                            for kc in range(2):
                                nc.tensor.matmul(
                                    at2[:],
                                    vp_aug[:, kc, b, h, :],
                                    ET[:, kc, :],
                                    start=(kc == 0),
                                    stop=(kc == 1),
                                )
                            rcp = p2.tile([DK, NSH], F32, name="rcp", tag="rcp",
                                          bufs=2)
                            nc.vector.reciprocal(rcp[:], at2[DK : 2 * DK, :])
                            nc.vector.tensor_tensor(
                                attn_sb[h * DK : (h + 1) * DK, :],
                                at2[0:DK, :],
                                rcp[:],
                                mybir.AluOpType.mult,
                            )
                        nc.sync.dma_start(a2a_in[b][nh, :, :], attn_sb[:])
                    nc.gpsimd.collective_compute(
                        "AllToAll",
                        mybir.AluOpType.bypass,
                        replica_groups=rg,
                        ins=[a2a_in[b][:]],
                        outs=[a2a_out[b][:]],
                    )
                    if b == 0:
                        # Wo arrives on the scalar queue during phase 2;
                        # needed when the first p3 batch starts.
                        for half in range(2):
                            hw = slice(half * (D // P // 2), (half + 1) * (D // P // 2))
                            nc.scalar.dma_start(Wo_sb[:, hw, :], Wo_p[:, hw, :])
                        if use_bo:
                            bo_sb = wpool.tile([P, D], F32)
                            nc.scalar.dma_start(bo_sb[:], bo_p[:])

                # ================= phase 3: output projection ==============
                for b in range(B):
                    gs = []
                    for dm in range(D // P):
                        g = p3.tile([P, NSH], BF16, name="g", tag="g", bufs=16)
                        nc.sync.dma_start(g[:], a2a_out[b][dm, :, :])
                        gs.append(g)
                    for mt in range(NSH // P):
                        f = [
                            p3ps.tile([P, 512], F32, name=f"f{fi}", tag="f", bufs=2)
                            for fi in range(2)
                        ]
                        for dm in range(D // P):
                            for fi in range(2):
                                nc.tensor.matmul(
                                    f[fi][:],
                                    gs[dm][:, mt * P : (mt + 1) * P],
                                    Wo_sb[:, dm, fi * 512 : (fi + 1) * 512],
                                    start=(dm == 0),
                                    stop=(dm == D // P - 1),
                                )
                        osb = p3.tile([P, D], F32, name="osb", tag="osb", bufs=3)
                        if use_bo:
                            for fi in range(2):
                                nc.vector.tensor_tensor(
                                    osb[:, fi * 512 : (fi + 1) * 512],
                                    f[fi][:],
                                    bo_sb[:, fi * 512 : (fi + 1) * 512],
                                    mybir.AluOpType.add,
                                )
                        else:
                            # split PSUM->SBUF eviction across DVE and Act
                            nc.vector.tensor_copy(osb[:, 0:512], f[0][:])
                            nc.scalar.copy(osb[:, 512:1024], f[1][:])
                        nc.scalar.dma_start(
                            out_p[b, mt * P : (mt + 1) * P, :], osb[:]
                        )

    return nc


def kernel(K, Q, V, We, be, Wf, bf, Wo, bo, n_heads, _trace=False):
    assert int(n_heads) == H
    K = np.asarray(K, np.float32)
    Q = np.asarray(Q, np.float32)
    V = np.asarray(V, np.float32)
    We = np.asarray(We, np.float32)
    be = np.asarray(be, np.float32)
    Wf = np.asarray(Wf, np.float32)
    bf = np.asarray(bf, np.float32)
    Wo = np.asarray(Wo, np.float32)
    bo = np.asarray(bo, np.float32)

    use_be = bool(np.any(be))
    use_bf = bool(np.any(bf))
    use_bo = bool(np.any(bo))

    key = (use_be, use_bf, use_bo)
    if key not in _BUILD_CACHE:
        _BUILD_CACHE[key] = _split_multi_waits(_build(*key))
    nc = _BUILD_CACHE[key]

    Kb = K.astype(NP_BF16)
    Vb = V.astype(NP_BF16)
    Qb = Q.astype(NP_BF16)
    WeS = np.ascontiguousarray(
        We.astype(NP_BF16).reshape(NCH, P, LK).transpose(1, 0, 2)
    )
    WfS = np.ascontiguousarray(
        Wf.astype(NP_BF16).reshape(NCH, P, LK).transpose(1, 0, 2)
    )
    WoS = np.ascontiguousarray(
        Wo.astype(NP_BF16).reshape(D // P, P, D).transpose(1, 0, 2)
    )

    in_maps = []
    for c in range(NC):
        cs = slice(P * c, P * (c + 1))
        m = {
            "Ks": np.ascontiguousarray(Kb[:, :, cs].transpose(1, 0, 2)),
            "Vs": np.ascontiguousarray(Vb[:, :, cs].transpose(1, 0, 2)),
            "QTs": np.ascontiguousarray(Qb[:, :, cs].transpose(0, 2, 1)),
            "WeS": WeS,
            "WfS": WfS,
            "WoS": WoS,
        }
        if use_be:
            m["beB"] = np.broadcast_to(be, (P, LK)).copy()
        if use_bf:
            m["bfB"] = np.ascontiguousarray(bf.reshape(2, P).T)
        if use_bo:
            m["boB"] = np.broadcast_to(bo, (P, D)).copy()
        in_maps.append(m)

    res = run_bass_kernel_spmd(nc, in_maps, list(range(NC)), trace=_trace)

    out = np.empty((B, N, D), np.float32)
    for c in range(NC):
        out[:, NSH * c : NSH * (c + 1), :] = res.results[c]["out"]
    if _trace:
        kernel._last_exec_time_ns = res.exec_time_ns
    return out


kernel._last_exec_time_ns = None


# revision 32
# speedup vs baseline: 1.0618x; 1.0618x over previous
"""MultiHeadLinearAttention (Linformer-style) on 8 trn2 NeuronCores.

Strategy (head-parallel attention + per-batch AllToAll + token-parallel
output projection):
  - 16 heads -> 8 cores: 2 heads (one d_model slice of 128) per core.
  - Per core, per batch b:
      Kp  [128(d2), 256k]  = K_slice^T @ We             (+be)
      Vp  [256(k), 128(d2)] = Wf^T @ V_slice            (+bf)
      per head h, 512-token block nh:
        s^T [256k, 512n] = Kp_h^T @ Q_h^T   (64-row PE tile, no padding)
        E^T = exp(s^T / 8)                  (one 1024-wide Act instr)
        at  [128, 512] = [Vp_h | ONES]^T @ E^T
            rows 0:64  = unnormalized attention output^T
            rows 64:128 = softmax denominator REPLICATED 64x (free PE
            broadcast -> plain elementwise normalize, no partition
            broadcast machinery)
        attn^T = at[0:64] * reciprocal(at[64:128])  (DVE recip + Pool mul)
  - Per-batch AllToAll (4 small collectives, pipelined with compute)
    exchanges attn^T n-chunks so core i ends with ALL d_model rows for
    tokens [512*i, 512*(i+1)) of that batch.
  - Per core, per b: out[n_shard] = attn_full^T.T @ Wo (+bo).

All matmuls in bf16 (host-cast) with fp32 PSUM accumulation.  Host
pre-arranges all DRAM tensors so every DMA is >=1KB-contiguous per
partition line, and DMAs are spread across the per-engine hardware DGE
queues (sync: K/V + a2a_out, vector: We/Wf + a2a_in, scalar: Q^T + Wo +
output stores) to avoid head-of-line blocking.
"""

import numpy as np
import ml_dtypes

import concourse.bass as bass
import concourse.mybir as mybir
from concourse.tile import TileContext
from concourse.bass_utils import run_bass_kernel_spmd

B, N, D, H, LK = 4, 4096, 1024, 16, 256
DK = D // H          # 64
NC = 8               # cores
NSH = N // NC        # 512 tokens per (core, nh-block)
P = 128
NCH = 32             # 128-row chunks of N
NPH = 8              # 512-col chunks of N

F32 = mybir.dt.float32
BF16 = mybir.dt.bfloat16
NP_BF16 = ml_dtypes.bfloat16

_BUILD_CACHE = {}

_ws_ctr = [0]


def _split_multi_waits(nc, lim=1):
    """Walrus codegen on this stack rejects instructions whose on_wait list
    exceeds the per-format wait-slot count ("Too many sync wait commands").
    Engines execute in order, so excess waits move onto preceding NOPs on
    the same engine with identical semantics."""
    for f in nc.m.functions:
        for blk in f.blocks:
            insts = blk.instructions
            if not any(
                ins.sync_info is not None and len(ins.sync_info.on_wait or []) > lim
                for ins in insts
            ):
                continue
            out = []
            for ins in insts:
                si = ins.sync_info
                waits = list(si.on_wait) if si is not None and si.on_wait else []
                if len(waits) > lim and ins.engine is not None:
                    keep = waits[-lim:]
                    rest = waits[:-lim]
                    while rest:
                        chunk, rest = rest[:lim], rest[lim:]
                        _ws_ctr[0] += 1
                        nop = mybir.InstNoOp(
                            name=f"I-waitsplit-{_ws_ctr[0]}", ins=[], outs=[]
                        )
                        nop.engine = ins.engine
                        nop.sync_info = mybir.SyncInfo(on_wait=chunk, on_update=[])
                        out.append(nop)
                    ins.sync_info = mybir.SyncInfo(
                        on_wait=keep, on_update=list(si.on_update or [])
                    )
                out.append(ins)
            blk.instructions = out
    return nc


def _build(use_be, use_bf, use_bo):
    nc = bass.Bass(num_devices=NC)

    Ks_p = nc.declare_dram_parameter("Ks", [N, B, P], BF16, isOutput=False)
    Vs_p = nc.declare_dram_parameter("Vs", [N, B, P], BF16, isOutput=False)
    QT_p = nc.declare_dram_parameter("QTs", [B, P, N], BF16, isOutput=False)
    We_p = nc.declare_dram_parameter("WeS", [P, NCH, LK], BF16, isOutput=False)
    Wf_p = nc.declare_dram_parameter("WfS", [P, NCH, LK], BF16, isOutput=False)
    Wo_p = nc.declare_dram_parameter("WoS", [P, D // P, D], BF16, isOutput=False)
    if use_be:
        be_p = nc.declare_dram_parameter("beB", [P, LK], F32, isOutput=False)
    if use_bf:
        bf_p = nc.declare_dram_parameter("bfB", [P, 2], F32, isOutput=False)
    if use_bo:
        bo_p = nc.declare_dram_parameter("boB", [P, D], F32, isOutput=False)
    out_p = nc.declare_dram_parameter("out", [B, NSH, D], F32, isOutput=True)

    rg = [list(range(NC))]

    with TileContext(nc) as tc:
        with (
            tc.tile_pool(name="wpool", bufs=1) as wpool,
            tc.tile_pool(name="state", bufs=1) as state,
            tc.tile_pool(name="dram", bufs=1, space="DRAM") as dram,
        ):
            # ---- persistent tiles
            We_sb = wpool.tile([P, NCH, LK], BF16)
            Wf_sb = wpool.tile([P, NCH, LK], BF16)
            Wo_sb = wpool.tile([P, D // P, D], BF16)
            # kp_sb[d2(=2 heads x 64), b, kc, k128]
            kp_sb = state.tile([P, B, 2, P], BF16)
            # vp_aug[k128, kc, b, h, (d64 | one)]: col 64 of ones makes the
            # at-matmul emit the softmax denominator as row 64 for free
            vp_aug = state.tile([P, 2, B, 2, DK + 1], BF16)
            nc.gpsimd.memset(vp_aug[:, :, :, :, DK : DK + 1], 1.0)

            # We/Wf chunks interleave with K/V loads on the sync queue so
            # the first phase-1 matmul can start after ~3 transfers.
            if use_be:
                be_sb = wpool.tile([P, LK], F32)
                nc.scalar.dma_start(be_sb[:], be_p[:])
            if use_bf:
                bf_sb = wpool.tile([P, 2], F32)
                nc.scalar.dma_start(bf_sb[:], bf_p[:])

            # ---- per-batch A2A buffers: 130 rows per destination =
            # [h0: 64 unnormalized numerator rows + 1 denominator row,
            #  h1: likewise].  Normalization happens after the exchange on
            # the token-sharded side where denominators can be batched.
            a2a_in = [
                dram.tile([NC, 2 * (DK + 1), NSH], BF16, name=f"a2a_in{b}")
                for b in range(B)
            ]
            a2a_out = [
                dram.tile([NC, 2 * (DK + 1), NSH], BF16, name=f"a2a_out{b}")
                for b in range(B)
            ]
            # reciprocal denominators staged in DRAM for the partition-
            # broadcast DMA (stride-0 source replication needs a DRAM src)
            rden_d = [
                dram.tile([2 * (D // P), NSH], BF16, name=f"rden{b}")
                for b in range(B)
            ]

            # ================= phase 1: Kp / Vp =================
            with (
                tc.tile_pool(name="p1", bufs=1) as p1,
                tc.tile_pool(name="p1ps", bufs=1, space="PSUM") as p1ps,
            ):
                kp_ps = [
                    p1ps.tile([P, LK], F32, name=f"kp{b}", tag=f"kp{b}")
                    for b in range(B)
                ]
                vp_ps = [
                    p1ps.tile([P, B * P], F32, name=f"vp{kc}", tag=f"vp{kc}")
                    for kc in range(2)
                ]
                cs = slice(0, 4)
                nc.sync.dma_start(We_sb[:, cs, :], We_p[:, cs, :])
                nc.sync.dma_start(Wf_sb[:, cs, :], Wf_p[:, cs, :])
                for ic2 in range(NCH // 2):
                    if ic2 % 2 == 0 and ic2 // 2 + 1 < NCH // 4:
                        # prefetch the next 4-chunk slab of We/Wf one slab
                        # ahead of its first use
                        ch = ic2 // 2 + 1
                        cs = slice(ch * 4, (ch + 1) * 4)
                        nc.sync.dma_start(We_sb[:, cs, :], We_p[:, cs, :])
                        nc.sync.dma_start(Wf_sb[:, cs, :], Wf_p[:, cs, :])
                    K4 = p1.tile([P, 2, B, P], BF16, name="K4", tag="K4", bufs=3)
                    nc.sync.dma_start(
                        K4[:],
                        Ks_p[ic2 * 2 * P : (ic2 + 1) * 2 * P, :, :].rearrange(
                            "(i p) b d -> p i b d", p=P
                        ),
                    )
                    V4 = p1.tile([P, 2, B, P], BF16, name="V4", tag="V4", bufs=3)
                    nc.sync.dma_start(
                        V4[:],
                        Vs_p[ic2 * 2 * P : (ic2 + 1) * 2 * P, :, :].rearrange(
                            "(i p) b d -> p i b d", p=P
                        ),
                    )
                    for i in range(2):
                        ic = ic2 * 2 + i
                        for b in range(B):
                            nc.tensor.matmul(
                                kp_ps[b][:],
                                K4[:, i, b, :],
                                We_sb[:, ic, :],
                                start=(ic == 0),
                                stop=(ic == NCH - 1),
                            )
                        for kc in range(2):
                            nc.tensor.matmul(
                                vp_ps[kc][:],
                                Wf_sb[:, ic, kc * P : (kc + 1) * P],
                                V4[:, i, :, :],
                                start=(ic == 0),
                                stop=(ic == NCH - 1),
                            )

                # epilogue: stage Kp/Vp into bf16 SBUF (Act engine; DVE is
                # the phase-2 pacing engine and Act is idle here)
                for b in range(B):
                    for kc in range(2):
                        ks = slice(kc * P, (kc + 1) * P)
                        if use_be:
                            nc.vector.tensor_tensor(
                                kp_sb[:, b, kc, :],
                                kp_ps[b][:, ks],
                                be_sb[:, ks],
                                mybir.AluOpType.add,
                            )
                        else:
                            nc.scalar.copy(kp_sb[:, b, kc, :], kp_ps[b][:, ks])
                    for kc in range(2):
                        for h in range(2):
                            src = vp_ps[kc][:, b * P + h * DK : b * P + (h + 1) * DK]
                            dst = vp_aug[:, kc, b, h, 0:DK]
                            if use_bf:
                                nc.vector.tensor_scalar_add(
                                    dst, src, bf_sb[:, kc : kc + 1]
                                )
                            else:
                                nc.scalar.copy(dst, src)

            # ================= phase 2: scores/softmax/attn + per-b A2A ====
            with (
                tc.tile_pool(name="p2", bufs=1) as p2,
                tc.tile_pool(name="p2ps", bufs=1, space="PSUM") as p2ps,
                tc.tile_pool(name="p3", bufs=1) as p3,
                tc.tile_pool(name="p3ps", bufs=1, space="PSUM") as p3ps,
            ):
                for b in range(B):
                    for nh in range(NPH):
                        if nh % 2 == 0:
                            QT2 = p2.tile(
                                [P, 2, NSH], BF16, name="QT2", tag="QT2", bufs=3
                            )
                            nc.scalar.dma_start(
                                QT2[:], QT_p[b, :, nh * NSH : (nh + 2) * NSH]
                            )
                        # staged as [65, h, n]; the a2a_in DMA's access
                        # pattern reorders to the [h*65+row] payload layout
                        attn_sb = p2.tile(
                            [DK + 1, 2, NSH], BF16, name="attn", tag="attn",
                            bufs=3
                        )
                        for h in range(2):
                            # head h of this core's 128-wide d_model slice
                            # lives on partitions h*64 .. h*64+64
                            hp = slice(h * DK, (h + 1) * DK)
                            st = p2ps.tile(
                                [P, 2, NSH], F32, name="st", tag="st", bufs=2
                            )
                            for kc in range(2):
                                nc.tensor.matmul(
                                    st[:, kc, :],
                                    kp_sb[hp, b, kc, :],
                                    QT2[hp, nh % 2, :],
                                    start=True,
                                    stop=True,
                                )
                            ET = p2.tile([P, 2, NSH], BF16, name="ET", tag="ET",
                                         bufs=2)
                            nc.scalar.activation(
                                ET[:, :, :],
                                st[:, :, :],
                                mybir.ActivationFunctionType.Exp,
                                scale=0.125,
                            )
                            # at rows 0:64 = unnormalized numerator^T,
                            # row 64 = softmax denominator (ones col)
                            at = p2ps.tile([DK + 1, NSH], F32, name="at",
                                           tag="at", bufs=2)
                            for kc in range(2):
                                nc.tensor.matmul(
                                    at[:],
                                    vp_aug[:, kc, b, h, :],
                                    ET[:, kc, :],
                                    start=(kc == 0),
                                    stop=(kc == 1),
                                )
                            # ship the whole 65-row block (num + den) bf16
                            nc.vector.tensor_copy(attn_sb[:, h, :], at[:])
                        nc.sync.dma_start(
                            a2a_in[b]
                            .rearrange("s (u v) f -> s v u f", v=DK + 1)[nh],
                            attn_sb[:],
                        )
                    nc.gpsimd.collective_compute(
                        "AllToAll",
                        mybir.AluOpType.bypass,
                        replica_groups=rg,
                        ins=[a2a_in[b][:]],
                        outs=[a2a_out[b][:]],
                    )
                    if b == 1:
                        # Wo arrives on the scalar queue during phase 2 —
                        # emitted after batch 1 so it doesn't outprioritize
                        # the early QT2 loads; needed when p3 b0 starts.
                        for half in range(2):
                            hw = slice(half * (D // P // 2), (half + 1) * (D // P // 2))
                            nc.scalar.dma_start(Wo_sb[:, hw, :], Wo_p[:, hw, :])
                        if use_bo:
                            bo_sb = wpool.tile([P, D], F32)
                            nc.scalar.dma_start(bo_sb[:], bo_p[:])

                # ================= phase 3: normalize + output projection ==
                for b in range(B):
                    # gather all 16 denominator rows (row 64 of each 65-row
                    # half-block from each source core) in one DMA
                    dview = a2a_out[b].rearrange("s (u v) f -> s u v f", v=DK + 1)
                    den_all = p3.tile([2 * (D // P), NSH], BF16, name="den",
                                      tag="den", bufs=2)
                    nc.sync.dma_start(den_all[:], dview[:, :, DK, :])
                    rden = p3.tile([2 * (D // P), NSH], BF16, name="rden",
                                   tag="rden", bufs=2)
                    with nc.allow_low_precision(
                        reason="bf16 reciprocal of softmax denominator; "
                        "matches the bf16 a2a payload precision"
                    ):
                        nc.vector.reciprocal(rden[:], den_all[:])
                    nc.sync.dma_start(rden_d[b][:], rden[:])
                    gns = []
                    for dm in range(D // P):
                        # partition-broadcast each head's reciprocal row to
                        # 64 partitions via stride-0 DRAM-source DMA
                        rb = p3.tile([P, NSH], BF16, name="rb", tag="rb",
                                     bufs=8)
                        for h in range(2):
                            nc.sync.dma_start(
                                rb[h * DK : (h + 1) * DK, :],
                                rden_d[b][2 * dm + h : 2 * dm + h + 1, :]
                                .broadcast_to([DK, NSH]),
                            )
                        # load just the 128 numerator rows (den row skipped
                        # via the strided payload view)
                        g = p3.tile([P, NSH], BF16, name="g", tag="g", bufs=16)
                        nc.sync.dma_start(g[:], dview[dm, :, 0:DK, :])
                        # normalize on the Pool engine (SBUF-only), leaving
                        # DVE/Act free for PSUM eviction
                        gn = p3.tile([P, NSH], BF16, name="gn", tag="gn",
                                     bufs=16)
                        for h in range(2):
                            nc.gpsimd.tensor_tensor(
                                gn[h * DK : (h + 1) * DK, :],
                                g[h * DK : (h + 1) * DK, :],
                                rb[h * DK : (h + 1) * DK, :],
                                mybir.AluOpType.mult,
                            )
                        gns.append(gn)
                    for mt in range(NSH // P):
                        f = [
                            p3ps.tile([P, 512], F32, name=f"f{fi}", tag="f",
                                      bufs=2)
                            for fi in range(2)
                        ]
                        for dm in range(D // P):
                            for fi in range(2):
                                nc.tensor.matmul(
                                    f[fi][:],
                                    gns[dm][:, mt * P : (mt + 1) * P],
                                    Wo_sb[:, dm, fi * 512 : (fi + 1) * 512],
                                    start=(dm == 0),
                                    stop=(dm == D // P - 1),
                                )
                        osb = p3.tile([P, D], F32, name="osb", tag="osb", bufs=3)
                        if use_bo:
                            for fi in range(2):
                                nc.vector.tensor_tensor(
                                    osb[:, fi * 512 : (fi + 1) * 512],
                                    f[fi][:],
                                    bo_sb[:, fi * 512 : (fi + 1) * 512],
                                    mybir.AluOpType.add,
                                )
                        else:
                            # split PSUM->SBUF eviction across DVE and Act
                            nc.vector.tensor_copy(osb[:, 0:512], f[0][:])
                            nc.scalar.copy(osb[:, 512:1024], f[1][:])
                        nc.scalar.dma_start(
                            out_p[b, mt * P : (mt + 1) * P, :], osb[:]
                        )

    return nc


def kernel(K, Q, V, We, be, Wf, bf, Wo, bo, n_heads, _trace=False):
    assert int(n_heads) == H
    K = np.asarray(K, np.float32)
    Q = np.asarray(Q, np.float32)
    V = np.asarray(V, np.float32)
    We = np.asarray(We, np.float32)
    be = np.asarray(be, np.float32)
    Wf = np.asarray(Wf, np.float32)
    bf = np.asarray(bf, np.float32)
    Wo = np.asarray(Wo, np.float32)
    bo = np.asarray(bo, np.float32)

    use_be = bool(np.any(be))
    use_bf = bool(np.any(bf))
    use_bo = bool(np.any(bo))

    key = (use_be, use_bf, use_bo)
    if key not in _BUILD_CACHE:
        _BUILD_CACHE[key] = _split_multi_waits(_build(*key))
    nc = _BUILD_CACHE[key]

    Kb = K.astype(NP_BF16)
    Vb = V.astype(NP_BF16)
    Qb = Q.astype(NP_BF16)
    WeS = np.ascontiguousarray(
        We.astype(NP_BF16).reshape(NCH, P, LK).transpose(1, 0, 2)
    )
    WfS = np.ascontiguousarray(
        Wf.astype(NP_BF16).reshape(NCH, P, LK).transpose(1, 0, 2)
    )
    WoS = np.ascontiguousarray(
        Wo.astype(NP_BF16).reshape(D // P, P, D).transpose(1, 0, 2)
    )

    in_maps = []
    for c in range(NC):
        cs = slice(P * c, P * (c + 1))
        m = {
            "Ks": np.ascontiguousarray(Kb[:, :, cs].transpose(1, 0, 2)),
            "Vs": np.ascontiguousarray(Vb[:, :, cs].transpose(1, 0, 2)),
            "QTs": np.ascontiguousarray(Qb[:, :, cs].transpose(0, 2, 1)),
            "WeS": WeS,
            "WfS": WfS,
            "WoS": WoS,
        }
        if use_be:
            m["beB"] = np.broadcast_to(be, (P, LK)).copy()
        if use_bf:
            m["bfB"] = np.ascontiguousarray(bf.reshape(2, P).T)
        if use_bo:
            m["boB"] = np.broadcast_to(bo, (P, D)).copy()
        in_maps.append(m)

    res = run_bass_kernel_spmd(nc, in_maps, list(range(NC)), trace=_trace)

    out = np.empty((B, N, D), np.float32)
    for c in range(NC):
        out[:, NSH * c : NSH * (c + 1), :] = res.results[c]["out"]
    if _trace:
        kernel._last_exec_time_ns = res.exec_time_ns
    return out


kernel._last_exec_time_ns = None


# revision 40
# speedup vs baseline: 1.0848x; 1.0217x over previous
"""MultiHeadLinearAttention (Linformer-style) on 8 trn2 NeuronCores.

Strategy (head-parallel attention + per-batch AllToAll + token-parallel
output projection):
  - 16 heads -> 8 cores: 2 heads (one d_model slice of 128) per core.
  - Per core, per batch b:
      Kp  [128(d2), 256k]  = K_slice^T @ We             (+be)
      Vp  [256(k), 128(d2)] = Wf^T @ V_slice            (+bf)
      per head h, 512-token block nh:
        s^T [256k, 512n] = Kp_h^T @ Q_h^T   (64-row PE tile, no padding)
        E^T = exp(s^T / 8)                  (one 1024-wide Act instr)
        at  [128, 512] = [Vp_h | ONES]^T @ E^T
            rows 0:64  = unnormalized attention output^T
            rows 64:128 = softmax denominator REPLICATED 64x (free PE
            broadcast -> plain elementwise normalize, no partition
            broadcast machinery)
        attn^T = at[0:64] * reciprocal(at[64:128])  (DVE recip + Pool mul)
  - Per-batch AllToAll (4 small collectives, pipelined with compute)
    exchanges attn^T n-chunks so core i ends with ALL d_model rows for
    tokens [512*i, 512*(i+1)) of that batch.
  - Per core, per b: out[n_shard] = attn_full^T.T @ Wo (+bo).

All matmuls in bf16 (host-cast) with fp32 PSUM accumulation.  Host
pre-arranges all DRAM tensors so every DMA is >=1KB-contiguous per
partition line, and DMAs are spread across the per-engine hardware DGE
queues (sync: K/V + a2a_out, vector: We/Wf + a2a_in, scalar: Q^T + Wo +
output stores) to avoid head-of-line blocking.
"""

import numpy as np
import ml_dtypes

import concourse.bass as bass
import concourse.mybir as mybir
from concourse.tile import TileContext
from concourse.bass_utils import run_bass_kernel_spmd
from concourse.tile_rust import add_dep_helper

B, N, D, H, LK = 4, 4096, 1024, 16, 256
DK = D // H          # 64
NC = 8               # cores
NSH = N // NC        # 512 tokens per (core, nh-block)
P = 128
NCH = 32             # 128-row chunks of N
NPH = 8              # 512-col chunks of N

F32 = mybir.dt.float32
BF16 = mybir.dt.bfloat16
NP_BF16 = ml_dtypes.bfloat16

_BUILD_CACHE = {}

_ws_ctr = [0]


def _split_multi_waits(nc, lim=1):
    """Walrus codegen on this stack rejects instructions whose on_wait list
    exceeds the per-format wait-slot count ("Too many sync wait commands").
    Engines execute in order, so excess waits move onto preceding NOPs on
    the same engine with identical semantics."""
    for f in nc.m.functions:
        for blk in f.blocks:
            insts = blk.instructions
            if not any(
                ins.sync_info is not None and len(ins.sync_info.on_wait or []) > lim
                for ins in insts
            ):
                continue
            out = []
            for ins in insts:
                si = ins.sync_info
                waits = list(si.on_wait) if si is not None and si.on_wait else []
                if len(waits) > lim and ins.engine is not None:
                    keep = waits[-lim:]
                    rest = waits[:-lim]
                    while rest:
                        chunk, rest = rest[:lim], rest[lim:]
                        _ws_ctr[0] += 1
                        nop = mybir.InstNoOp(
                            name=f"I-waitsplit-{_ws_ctr[0]}", ins=[], outs=[]
                        )
                        nop.engine = ins.engine
                        nop.sync_info = mybir.SyncInfo(on_wait=chunk, on_update=[])
                        out.append(nop)
                    ins.sync_info = mybir.SyncInfo(
                        on_wait=keep, on_update=list(si.on_update or [])
                    )
                out.append(ins)
            blk.instructions = out
    return nc


def _build(use_be, use_bf, use_bo):
    nc = bass.Bass(num_devices=NC)

    Ks_p = nc.declare_dram_parameter("Ks", [N, B, P], BF16, isOutput=False)
    Vs_p = nc.declare_dram_parameter("Vs", [N, B, P], BF16, isOutput=False)
    QT_p = nc.declare_dram_parameter("QTs", [B, P, N], BF16, isOutput=False)
    We_p = nc.declare_dram_parameter("WeS", [P, NCH, LK], BF16, isOutput=False)
    Wf_p = nc.declare_dram_parameter("WfS", [P, NCH, LK], BF16, isOutput=False)
    Wo_p = nc.declare_dram_parameter("WoS", [P, D // P, D], BF16, isOutput=False)
    if use_be:
        be_p = nc.declare_dram_parameter("beB", [P, LK], F32, isOutput=False)
    if use_bf:
        bf_p = nc.declare_dram_parameter("bfB", [P, 2], F32, isOutput=False)
    if use_bo:
        bo_p = nc.declare_dram_parameter("boB", [P, D], F32, isOutput=False)
    out_p = nc.declare_dram_parameter("out", [B, NSH, D], F32, isOutput=True)

    rg = [list(range(NC))]

    with TileContext(nc) as tc:
        with (
            tc.tile_pool(name="wpool", bufs=1) as wpool,
            tc.tile_pool(name="state", bufs=1) as state,
            tc.tile_pool(name="dram", bufs=1, space="DRAM") as dram,
        ):
            # ---- persistent tiles
            We_sb = wpool.tile([P, NCH, LK], BF16)
            Wf_sb = wpool.tile([P, NCH, LK], BF16)
            Wo_sb = wpool.tile([P, D // P, D], BF16)
            # kp_sb[d2(=2 heads x 64), b, kc, k128]
            kp_sb = state.tile([P, B, 2, P], BF16)
            # vp_aug[k128, kc, b, h, (d64 | one)]: col 64 of ones makes the
            # at-matmul emit the softmax denominator as row 64 for free
            vp_aug = state.tile([P, 2, B, 2, DK + 1], BF16)
            nc.gpsimd.memset(vp_aug[:, :, :, :, DK : DK + 1], 1.0)

            # We/Wf chunks interleave with K/V loads on the sync queue so
            # the first phase-1 matmul can start after ~3 transfers.
            if use_be:
                be_sb = wpool.tile([P, LK], F32)
                nc.scalar.dma_start(be_sb[:], be_p[:])
            if use_bf:
                bf_sb = wpool.tile([P, 2], F32)
                nc.scalar.dma_start(bf_sb[:], bf_p[:])

            # ---- per-batch A2A buffers: 130 rows per destination =
            # [h0: 64 unnormalized numerator rows + 1 denominator row,
            #  h1: likewise].  Normalization happens after the exchange on
            # the token-sharded side where denominators can be batched.
            a2a_in = [
                dram.tile([NC, 2 * (DK + 1), NSH], BF16, name=f"a2a_in{b}")
                for b in range(B)
            ]
            a2a_out = [
                dram.tile([NC, 2 * (DK + 1), NSH], BF16, name=f"a2a_out{b}")
                for b in range(B)
            ]
            # reciprocal denominators staged in DRAM for the partition-
            # broadcast DMA (stride-0 source replication needs a DRAM src)
            rden_d = [
                dram.tile([2 * (D // P), NSH], BF16, name=f"rden{b}")
                for b in range(B)
            ]

            # ================= phase 1: Kp / Vp =================
            with (
                tc.tile_pool(name="p1", bufs=1) as p1,
                tc.tile_pool(name="p1ps", bufs=1, space="PSUM") as p1ps,
            ):
                kp_ps = [
                    p1ps.tile([P, LK], F32, name=f"kp{b}", tag=f"kp{b}")
                    for b in range(B)
                ]
                vp_ps = [
                    p1ps.tile([P, B * P], F32, name=f"vp{kc}", tag=f"vp{kc}")
                    for kc in range(2)
                ]
                cs = slice(0, 4)
                nc.sync.dma_start(We_sb[:, cs, :], We_p[:, cs, :])
                nc.sync.dma_start(Wf_sb[:, cs, :], Wf_p[:, cs, :])
                for ic2 in range(NCH // 2):
                    if ic2 % 2 == 0 and ic2 // 2 + 1 < NCH // 4:
                        # prefetch the next 4-chunk slab of We/Wf one slab
                        # ahead of its first use
                        ch = ic2 // 2 + 1
                        cs = slice(ch * 4, (ch + 1) * 4)
                        nc.sync.dma_start(We_sb[:, cs, :], We_p[:, cs, :])
                        nc.sync.dma_start(Wf_sb[:, cs, :], Wf_p[:, cs, :])
                    K4 = p1.tile([P, 2, B, P], BF16, name="K4", tag="K4", bufs=3)
                    nc.sync.dma_start(
                        K4[:],
                        Ks_p[ic2 * 2 * P : (ic2 + 1) * 2 * P, :, :].rearrange(
                            "(i p) b d -> p i b d", p=P
                        ),
                    )
                    V4 = p1.tile([P, 2, B, P], BF16, name="V4", tag="V4", bufs=3)
                    nc.sync.dma_start(
                        V4[:],
                        Vs_p[ic2 * 2 * P : (ic2 + 1) * 2 * P, :, :].rearrange(
                            "(i p) b d -> p i b d", p=P
                        ),
                    )
                    for i in range(2):
                        ic = ic2 * 2 + i
                        for b in range(B):
                            nc.tensor.matmul(
                                kp_ps[b][:],
                                K4[:, i, b, :],
                                We_sb[:, ic, :],
                                start=(ic == 0),
                                stop=(ic == NCH - 1),
                            )
                        for kc in range(2):
                            nc.tensor.matmul(
                                vp_ps[kc][:],
                                Wf_sb[:, ic, kc * P : (kc + 1) * P],
                                V4[:, i, :, :],
                                start=(ic == 0),
                                stop=(ic == NCH - 1),
                            )

                # epilogue: stage Kp/Vp into bf16 SBUF (Act engine; DVE is
                # the phase-2 pacing engine and Act is idle here)
                for b in range(B):
                    for kc in range(2):
                        ks = slice(kc * P, (kc + 1) * P)
                        if use_be:
                            nc.vector.tensor_tensor(
                                kp_sb[:, b, kc, :],
                                kp_ps[b][:, ks],
                                be_sb[:, ks],
                                mybir.AluOpType.add,
                            )
                        else:
                            nc.scalar.copy(kp_sb[:, b, kc, :], kp_ps[b][:, ks])
                    for kc in range(2):
                        for h in range(2):
                            src = vp_ps[kc][:, b * P + h * DK : b * P + (h + 1) * DK]
                            dst = vp_aug[:, kc, b, h, 0:DK]
                            if use_bf:
                                nc.vector.tensor_scalar_add(
                                    dst, src, bf_sb[:, kc : kc + 1]
                                )
                            else:
                                nc.scalar.copy(dst, src)

            # ================= phase 2: scores/softmax/attn + per-b A2A ====
            with (
                tc.tile_pool(name="p2", bufs=1) as p2,
                tc.tile_pool(name="p2ps", bufs=1, space="PSUM") as p2ps,
                tc.tile_pool(name="p3", bufs=1) as p3,
                tc.tile_pool(name="p3ps", bufs=1, space="PSUM") as p3ps,
            ):
                last_at = [None] * B    # last at-matmul of each p2 batch
                last_write = [None] * B  # last a2a_in write of each batch
                cc_ins = [None] * B      # collective trigger of each batch
                for b in range(B):
                    for nh in range(NPH):
                        if nh % 2 == 0:
                            QT2 = p2.tile(
                                [P, 2, NSH], BF16, name="QT2", tag="QT2", bufs=3
                            )
                            nc.scalar.dma_start(
                                QT2[:], QT_p[b, :, nh * NSH : (nh + 2) * NSH]
                            )
                        # staged as [65, h, n]; the a2a_in DMA's access
                        # pattern reorders to the [h*65+row] payload layout
                        attn_sb = p2.tile(
                            [DK + 1, 2, NSH], BF16, name="attn", tag="attn",
                            bufs=3
                        )
                        for h in range(2):
                            # head h of this core's 128-wide d_model slice
                            # lives on partitions h*64 .. h*64+64
                            hp = slice(h * DK, (h + 1) * DK)
                            st = p2ps.tile(
                                [P, 2, NSH], F32, name="st", tag="st", bufs=2
                            )
                            for kc in range(2):
                                nc.tensor.matmul(
                                    st[:, kc, :],
                                    kp_sb[hp, b, kc, :],
                                    QT2[hp, nh % 2, :],
                                    start=True,
                                    stop=True,
                                )
                            # exp split per kc half so the at-matmul can
                            # start after the first half (finer Act/PE
                            # pipelining keeps the PE denser)
                            ET = p2.tile([P, 2, NSH], BF16, name="ET", tag="ET",
                                         bufs=2)
                            for kc in range(2):
                                nc.scalar.activation(
                                    ET[:, kc, :],
                                    st[:, kc, :],
                                    mybir.ActivationFunctionType.Exp,
                                    scale=0.125,
                                )
                            # at rows 0:64 = unnormalized numerator^T,
                            # row 64 = softmax denominator (ones col)
                            at = p2ps.tile([DK + 1, NSH], F32, name="at",
                                           tag="at", bufs=2)
                            for kc in range(2):
                                mm = nc.tensor.matmul(
                                    at[:],
                                    vp_aug[:, kc, b, h, :],
                                    ET[:, kc, :],
                                    start=(kc == 0),
                                    stop=(kc == 1),
                                )
                            last_at[b] = mm
                            # ship the whole 65-row block (num + den) bf16
                            nc.vector.tensor_copy(attn_sb[:, h, :], at[:])
                        last_write[b] = nc.sync.dma_start(
                            a2a_in[b]
                            .rearrange("s (u v) f -> s v u f", v=DK + 1)[nh],
                            attn_sb[:],
                        )
                    cc_ins[b] = nc.gpsimd.collective_compute(
                        "AllToAll",
                        mybir.AluOpType.bypass,
                        replica_groups=rg,
                        ins=[a2a_in[b][:]],
                        outs=[a2a_out[b][:]],
                    )
                    if b == 1:
                        # Wo arrives on the scalar queue during phase 2 —
                        # emitted after batch 1 so it doesn't outprioritize
                        # the early QT2 loads; needed when p3 b0 starts.
                        for half in range(2):
                            hw = slice(half * (D // P // 2), (half + 1) * (D // P // 2))
                            nc.scalar.dma_start(Wo_sb[:, hw, :], Wo_p[:, hw, :])
                        if use_bo:
                            bo_sb = wpool.tile([P, D], F32)
                            nc.scalar.dma_start(bo_sb[:], bo_p[:])

                # ================= phase 3: normalize + output projection ==
                for b in range(B):
                    # Pin p3(b)'s sync-queue DMAs behind p2(b+2)'s last
                    # a2a_in write: a descriptor waiting on a slow AllToAll
                    # would head-of-line block later a2a_in writes on the
                    # same hardware queue (and so the next collectives).
                    sync_anchor = last_write[min(b + 2, B - 1)]
                    pool_anchor = cc_ins[min(b + 2, B - 1)]
                    # gather all 16 denominator rows (row 64 of each 65-row
                    # half-block from each source core) in one DMA
                    dview = a2a_out[b].rearrange("s (u v) f -> s u v f", v=DK + 1)
                    den_all = p3.tile([2 * (D // P), NSH], BF16, name="den",
                                      tag="den", bufs=2)
                    dg = nc.sync.dma_start(den_all[:], dview[:, :, DK, :])
                    add_dep_helper(dg.ins, sync_anchor.ins, sync=False,
                                   reason="order p3 DMAs after p2 writes")
                    rden = p3.tile([2 * (D // P), NSH], BF16, name="rden",
                                   tag="rden", bufs=2)
                    with nc.allow_low_precision(
                        reason="bf16 reciprocal of softmax denominator; "
                        "matches the bf16 a2a payload precision"
                    ):
                        nc.vector.reciprocal(rden[:], den_all[:])
                    nc.sync.dma_start(rden_d[b][:], rden[:])
                    gns = []
                    for dm in range(D // P):
                        # partition-broadcast each head's reciprocal row to
                        # 64 partitions via stride-0 DRAM-source DMA
                        rb = p3.tile([P, NSH], BF16, name="rb", tag="rb",
                                     bufs=8)
                        for h in range(2):
                            nc.sync.dma_start(
                                rb[h * DK : (h + 1) * DK, :],
                                rden_d[b][2 * dm + h : 2 * dm + h + 1, :]
                                .broadcast_to([DK, NSH]),
                            )
                        # load just the 128 numerator rows (den row skipped
                        # via the strided payload view)
                        g = p3.tile([P, NSH], BF16, name="g", tag="g", bufs=16)
                        gld = nc.sync.dma_start(g[:], dview[dm, :, 0:DK, :])
                        add_dep_helper(gld.ins, sync_anchor.ins, sync=False,
                                       reason="order g loads after p2 writes")
                        # normalize split across Pool (h0) and DVE (h1) —
                        # Pool tensor ops measured ~1.15us for [64,512]
                        gn = p3.tile([P, NSH], BF16, name="gn", tag="gn",
                                     bufs=16)
                        for h, eng in ((0, nc.gpsimd), (1, nc.vector)):
                            mi = eng.tensor_tensor(
                                gn[h * DK : (h + 1) * DK, :],
                                g[h * DK : (h + 1) * DK, :],
                                rb[h * DK : (h + 1) * DK, :],
                                mybir.AluOpType.mult,
                            )
                            if h == 0:
                                # keep Pool's stream clear of work that waits
                                # on collectives until later triggers fired
                                add_dep_helper(
                                    mi.ins, pool_anchor.ins, sync=False,
                                    reason="order Pool mults after triggers",
                                )
                        gns.append(gn)
                    for mt in range(NSH // P):
                        f = [
                            p3ps.tile([P, 512], F32, name=f"f{fi}", tag="f",
                                      bufs=2)
                            for fi in range(2)
                        ]
                        for dm in range(D // P):
                            for fi in range(2):
                                mm = nc.tensor.matmul(
                                    f[fi][:],
                                    gns[dm][:, mt * P : (mt + 1) * P],
                                    Wo_sb[:, dm, fi * 512 : (fi + 1) * 512],
                                    start=(dm == 0),
                                    stop=(dm == D // P - 1),
                                )
                                if dm == 0 and fi == 0 and mt == 0:
                                    # pin p3(b)'s PE work behind p2(b+2)'s:
                                    # the scheduler's CC-latency estimate is
                                    # optimistic, and an early f-matmul
                                    # waiting on a slow AllToAll head-of-line
                                    # blocks the in-order PE queue.
                                    anchor = last_at[min(b + 2, B - 1)]
                                    add_dep_helper(
                                        mm.ins,
                                        anchor.ins,
                                        sync=False,
                                        reason="order p3 PE after p2(b+2)",
                                    )
                        osb = p3.tile([P, D], F32, name="osb", tag="osb", bufs=3)
                        if use_bo:
                            for fi in range(2):
                                nc.vector.tensor_tensor(
                                    osb[:, fi * 512 : (fi + 1) * 512],
                                    f[fi][:],
                                    bo_sb[:, fi * 512 : (fi + 1) * 512],
                                    mybir.AluOpType.add,
                                )
                        else:
                            # split PSUM->SBUF eviction across DVE and Act
                            nc.vector.tensor_copy(osb[:, 0:512], f[0][:])
                            nc.scalar.copy(osb[:, 512:1024], f[1][:])
                        nc.scalar.dma_start(
                            out_p[b, mt * P : (mt + 1) * P, :], osb[:]
                        )

    return nc


def kernel(K, Q, V, We, be, Wf, bf, Wo, bo, n_heads, _trace=False):
    assert int(n_heads) == H
    K = np.asarray(K, np.float32)
    Q = np.asarray(Q, np.float32)
    V = np.asarray(V, np.float32)
    We = np.asarray(We, np.float32)
    be = np.asarray(be, np.float32)
    Wf = np.asarray(Wf, np.float32)
    bf = np.asarray(bf, np.float32)
    Wo = np.asarray(Wo, np.float32)
    bo = np.asarray(bo, np.float32)

    use_be = bool(np.any(be))
    use_bf = bool(np.any(bf))
    use_bo = bool(np.any(bo))

    key = (use_be, use_bf, use_bo)
    if key not in _BUILD_CACHE:
        _BUILD_CACHE[key] = _split_multi_waits(_build(*key))
    nc = _BUILD_CACHE[key]

    Kb = K.astype(NP_BF16)
    Vb = V.astype(NP_BF16)
    Qb = Q.astype(NP_BF16)
    WeS = np.ascontiguousarray(
        We.astype(NP_BF16).reshape(NCH, P, LK).transpose(1, 0, 2)
    )
    WfS = np.ascontiguousarray(
        Wf.astype(NP_BF16).reshape(NCH, P, LK).transpose(1, 0, 2)
    )
    WoS = np.ascontiguousarray(
        Wo.astype(NP_BF16).reshape(D // P, P, D).transpose(1, 0, 2)
    )

    in_maps = []
    for c in range(NC):
        cs = slice(P * c, P * (c + 1))
        m = {
            "Ks": np.ascontiguousarray(Kb[:, :, cs].transpose(1, 0, 2)),
            "Vs": np.ascontiguousarray(Vb[:, :, cs].transpose(1, 0, 2)),
            "QTs": np.ascontiguousarray(Qb[:, :, cs].transpose(0, 2, 1)),
            "WeS": WeS,
            "WfS": WfS,
            "WoS": WoS,
        }
        if use_be:
            m["beB"] = np.broadcast_to(be, (P, LK)).copy()
        if use_bf:
            m["bfB"] = np.ascontiguousarray(bf.reshape(2, P).T)
        if use_bo:
            m["boB"] = np.broadcast_to(bo, (P, D)).copy()
        in_maps.append(m)

    res = run_bass_kernel_spmd(nc, in_maps, list(range(NC)), trace=_trace)

    out = np.empty((B, N, D), np.float32)
    for c in range(NC):
        out[:, NSH * c : NSH * (c + 1), :] = res.results[c]["out"]
    if _trace:
        kernel._last_exec_time_ns = res.exec_time_ns
    return out


kernel._last_exec_time_ns = None
